# revision 1
# baseline (speedup 1.0000x reference)
"""Pairwise-interaction kernel for Trainium2 (raw Bass), 8-core SPMD.

Computes out[b, p, :] = x[b, i(p), :] * x[b, j(p), :] for all pairs
(i < j) of the F=26 feature rows, p ordered row-major (i outer, j inner).

Sharding: data-parallel over the batch dim (16384 -> 8 x 2048), no
cross-core communication. Per core: 16 tiles of 128 samples on SBUF
partitions. For each tile the "i" row is broadcast (stride-0 AP) against
the contiguous tail x[:, i+1:] with one fp32 tensor_tensor multiply per
i (25 per tile), writing a compact [128, 10400] output tile stored with
a single 5.3MB DMA.

Raw-Bass sync scheme (every instruction carries at most ONE semaphore
wait — the ISA allows exactly one wait slot per instruction):
  sem_ld  (+16 per load DMA, scalar/ACT HWDGE ring)
  sem_st  (+16 per store DMA, sync/SP HWDGE ring)
  sem_tt  (+1 by the last TT of each tile, vector engine)
  loads   wait sem_tt >= t-XB+1   (previous tenant's TTs have read the slot)
  vector  waits sem_ld >= 16(t+1) and sem_st >= 16(t-YB+1) as standalone
          wait ops, then runs the 25 TTs wait-free
  stores  wait sem_tt >= t+1      (this tile's TTs are done)
"""

import numpy as np

import concourse.bass as bass
from concourse import mybir
from concourse.bass_utils import run_bass_kernel_spmd

B, F, D = 16384, 26, 32
NCORES = 8
BC = B // NCORES           # 2048 samples per core
P = 128                    # SBUF partitions per tile
NT = BC // P               # 16 tiles per core
FD = F * D                 # 832
NPAIR = F * (F - 1) // 2   # 325
OD = NPAIR * D             # 10400

XB = 3                     # input tile buffers
YB = 2                     # output tile buffers

# Chunked stores: split each tile's 325 pair-rows into 4 chunks of
# consecutive i-blocks so the store of a chunk overlaps the compute of the
# next one (shrinks pipeline fill/drain). Chunks are (i_lo, i_hi) ranges.
CHUNKS = [(0, 3), (3, 7), (7, 12), (12, 25)]
NCH = len(CHUNKS)


def _chunk_pair_off(i_lo):
    # first output pair index for block i = i_lo
    return sum(F - 1 - i for i in range(i_lo))

F32 = mybir.dt.float32

_nc_cache = None


def _build_nc():
    nc = bass.Bass()
    x = nc.declare_dram_parameter("x", [BC, FD], F32, isOutput=False)
    y = nc.declare_dram_parameter("y", [BC, OD], F32, isOutput=True)
    xv = x[:].rearrange("(n p) m -> n p m", p=P)
    yv = y[:].rearrange("(n p) m -> n p m", p=P)

    with (
        nc.sbuf_tensor([P, XB * FD], F32) as xbuf,
        nc.sbuf_tensor([P, YB * OD], F32) as ybuf,
        nc.semaphore("sem_ld") as sem_ld,
        nc.semaphore("sem_st") as sem_st,
        nc.semaphore("sem_tt") as sem_tt,
        nc.Block() as blk,
    ):
        xts = [xbuf[:, b * FD : (b + 1) * FD] for b in range(XB)]
        yts = [ybuf[:, b * OD : (b + 1) * OD] for b in range(YB)]

        @blk.scalar
        def _(scalar):
            for t in range(NT):
                ld = scalar.dma_start(xts[t % XB], xv[t])
                if t >= XB:
                    # Slot free once the previous tenant's TTs have read it
                    # (sem_tt counts chunk completions, NCH per tile).
                    ld._wait_ge(sem_tt, NCH * (t - XB + 1))
                ld.then_inc(sem_ld, 16)

        @blk.sync
        def _(sync):
            for t in range(NT):
                for c, (i_lo, i_hi) in enumerate(CHUNKS):
                    p_lo = _chunk_pair_off(i_lo)
                    p_hi = _chunk_pair_off(i_hi)
                    st = sync.dma_start(
                        yv[t][:, p_lo * D : p_hi * D],
                        yts[t % YB][:, p_lo * D : p_hi * D],
                    )
                    st._wait_ge(sem_tt, NCH * t + c + 1)
                    st.then_inc(sem_st, 16)

        @blk.vector
        def _(v):
            for t in range(NT):
                xt = xts[t % XB]
                yt = yts[t % YB]
                v.wait_ge(sem_ld, 16 * (t + 1))
                for c, (i_lo, i_hi) in enumerate(CHUNKS):
                    if t >= YB:
                        # chunk c of tile t-YB has been stored
                        v.wait_ge(sem_st, 16 * (NCH * (t - YB) + c + 1))
                    off = _chunk_pair_off(i_lo)
                    for i in range(i_lo, i_hi):
                        nrep = F - 1 - i
                        in0 = (
                            xt[:, i * D : (i + 1) * D]
                            .unsqueeze(1)
                            .broadcast_to([P, nrep, D])
                        )
                        in1 = xt[:, (i + 1) * D : FD].rearrange(
                            "p (r d) -> p r d", d=D
                        )
                        outap = yt[:, off * D : (off + nrep) * D].rearrange(
                            "p (r d) -> p r d", d=D
                        )
                        tt = nc.vector.tensor_mul(outap, in0, in1)
                        off += nrep
                    tt.then_inc(sem_tt, 1)

    return nc


def kernel(inputs: np.ndarray) -> np.ndarray:
    global _nc_cache
    if _nc_cache is None:
        _nc_cache = _build_nc()
    nc = _nc_cache

    x = np.ascontiguousarray(np.asarray(inputs, dtype=np.float32)).reshape(B, FD)
    shards = x.reshape(NCORES, BC, FD)
    in_maps = [{"x": shards[c]} for c in range(NCORES)]
    res = run_bass_kernel_spmd(nc, in_maps, list(range(NCORES)))
    out = np.concatenate(
        [res.results[c]["y"].reshape(BC, NPAIR, D) for c in range(NCORES)], axis=0
    )
    return out



# revision 2
# speedup vs baseline: 1.5675x; 1.5675x over previous
"""Pairwise-interaction kernel for Trainium2 (raw Bass), 8-core SPMD.

Computes out[b, p, :] = x[b, i(p), :] * x[b, j(p), :] for all pairs
(i < j) of the F=26 feature rows, p ordered row-major (i outer, j inner).

Sharding: data-parallel over the batch dim (16384 -> 8 x 2048), no
cross-core communication.

v2 design notes (vs the f32 v1 baseline at ~221us):
  * The v1 kernel was vector-engine bound (DVE ~90% busy, f32
    tensor_tensor runs at 1 elem/cyc/partition). All tensors are now
    bf16: DVE 2x packing mode doubles TT throughput AND the HBM store
    shrinks from 85MB to 43MB per core. The extra rounding error
    (~1.2% worst case: two input roundings + one output rounding at
    2^-8 each) is far inside the 2e-2 relative-error gate; f32<->bf16
    conversion happens on the host.
  * G sample-groups share one supertile so each of the 25 TT
    instructions covers G*(25-i)*32 elements, amortizing the ~58-cycle
    per-instruction DVE bubble.
  * Output stores are split across BOTH HWDGE rings (sync + scalar;
    the only two on TRN2) so neither ring has to carry the full 43MB.
    Loads (3.4MB total) ride on the scalar ring.

Raw-Bass sync scheme (every instruction carries at most ONE semaphore
wait; extra ordering uses standalone wait_ge ops on the engine queue):
  sem_ld   (+16 per load DMA, scalar ring)
  sem_st_s (+16 per store DMA on the sync ring: chunks 0 and 3)
  sem_st_a (+16 per store DMA on the scalar ring: chunks 1 and 2)
  sem_tt   (+1 by the last TT of each chunk, vector engine)
"""

import numpy as np
import ml_dtypes

import concourse.bass as bass
from concourse import mybir
from concourse.bass_utils import run_bass_kernel_spmd

B, F, D = 16384, 26, 32
NCORES = 8
BC = B // NCORES           # 2048 samples per core
P = 128                    # SBUF partitions per tile
G = 2                      # sample groups per supertile
NTS = BC // (P * G)        # supertiles per core
FD = F * D                 # 832
NPAIR = F * (F - 1) // 2   # 325
OD = NPAIR * D             # 10400

XB = 3                     # input supertile buffers
YB = 2                     # output supertile buffers

# Chunked stores: split each supertile's 25 i-blocks into 4 chunks so
# stores overlap compute. Chunks 0,3 go to the sync ring, 1,2 to the
# scalar ring (pair counts 94/78/75/78; ring bytes balance to ~49/51
# once the loads on the scalar ring are counted).
CHUNKS = [(0, 3), (3, 7), (7, 12), (12, 25)]
NCH = len(CHUNKS)
SYNC_CHUNKS = (0, 3)
SCAL_CHUNKS = (1, 2)

BF16 = mybir.dt.bfloat16
NP_BF16 = ml_dtypes.bfloat16


def _pair_off(i_lo):
    # first output pair index for block i = i_lo
    return sum(F - 1 - i for i in range(i_lo))


_nc_cache = None


def _build_nc():
    nc = bass.Bass()
    x = nc.declare_dram_parameter("x", [BC, FD], BF16, isOutput=False)
    y = nc.declare_dram_parameter("y", [BC, OD], BF16, isOutput=True)
    # sample s = t*G*P + g*P + p  ->  dims ordered (t, p, g, m) so the
    # DMA iteration order matches the SBUF-side [p, g, m] views.
    xv = x[:].rearrange("(t g p) m -> t p g m", g=G, p=P)
    yv = y[:].rearrange("(t g p) m -> t p g m", g=G, p=P)

    with (
        nc.sbuf_tensor([P, XB * G * FD], BF16) as xbuf,
        nc.sbuf_tensor([P, YB * G * OD], BF16) as ybuf,
        nc.semaphore("sem_ld") as sem_ld,
        nc.semaphore("sem_st_s") as sem_st_s,
        nc.semaphore("sem_st_a") as sem_st_a,
        nc.semaphore("sem_tt") as sem_tt,
        nc.Block() as blk,
    ):
        # [P, g, m] views of each buffer slot
        xts = [
            xbuf[:, b * G * FD : (b + 1) * G * FD].rearrange(
                "p (g m) -> p g m", g=G
            )
            for b in range(XB)
        ]
        yts = [
            ybuf[:, b * G * OD : (b + 1) * G * OD].rearrange(
                "p (g m) -> p g m", g=G
            )
            for b in range(YB)
        ]

        def store(eng, t, c):
            i_lo, i_hi = CHUNKS[c]
            p_lo, p_hi = _pair_off(i_lo), _pair_off(i_hi)
            st = eng.dma_start(
                yv[t][:, :, p_lo * D : p_hi * D],
                yts[t % YB][:, :, p_lo * D : p_hi * D],
            )
            st._wait_ge(sem_tt, NCH * t + c + 1)
            return st

        @blk.scalar
        def _(scalar):
            for t in range(NTS):
                if t >= XB:
                    # slot free once the previous tenant's TTs have read it
                    scalar.wait_ge(sem_tt, NCH * (t - XB + 1))
                scalar.dma_start(xts[t % XB], xv[t]).then_inc(sem_ld, 16)
                if t > 0:
                    for c in SCAL_CHUNKS:
                        store(scalar, t - 1, c).then_inc(sem_st_a, 16)
            for c in SCAL_CHUNKS:
                store(scalar, NTS - 1, c).then_inc(sem_st_a, 16)

        @blk.sync
        def _(sync):
            for t in range(NTS):
                for c in SYNC_CHUNKS:
                    store(sync, t, c).then_inc(sem_st_s, 16)

        @blk.vector
        def _(v):
            for t in range(NTS):
                xt = xts[t % XB]
                yt = yts[t % YB]
                v.wait_ge(sem_ld, 16 * (t + 1))
                for c, (i_lo, i_hi) in enumerate(CHUNKS):
                    if t >= YB:
                        # chunk c of supertile t-YB has left the ybuf slot;
                        # each ring stores its chunks in (t, c) order
                        if c in SYNC_CHUNKS:
                            n = 2 * (t - YB) + SYNC_CHUNKS.index(c) + 1
                            v.wait_ge(sem_st_s, 16 * n)
                        else:
                            n = 2 * (t - YB) + SCAL_CHUNKS.index(c) + 1
                            v.wait_ge(sem_st_a, 16 * n)
                    off = _pair_off(i_lo)
                    for i in range(i_lo, i_hi):
                        nrep = F - 1 - i
                        in0 = (
                            xt[:, :, i * D : (i + 1) * D]
                            .unsqueeze(2)
                            .broadcast_to([P, G, nrep, D])
                        )
                        in1 = xt[:, :, (i + 1) * D : FD].rearrange(
                            "p g (r d) -> p g r d", d=D
                        )
                        outap = yt[
                            :, :, off * D : (off + nrep) * D
                        ].rearrange("p g (r d) -> p g r d", d=D)
                        tt = nc.vector.tensor_mul(outap, in0, in1)
                        off += nrep
                    tt.then_inc(sem_tt, 1)

    return nc


def _make_in_maps(inputs: np.ndarray):
    x = np.asarray(inputs, dtype=np.float32).reshape(B, FD).astype(NP_BF16)
    shards = np.ascontiguousarray(x.reshape(NCORES, BC, FD))
    return [{"x": shards[c]} for c in range(NCORES)]


def kernel(inputs: np.ndarray) -> np.ndarray:
    global _nc_cache
    if _nc_cache is None:
        _nc_cache = _build_nc()
    nc = _nc_cache

    in_maps = _make_in_maps(inputs)
    res = run_bass_kernel_spmd(nc, in_maps, list(range(NCORES)))
    out = np.concatenate([res.results[c]["y"] for c in range(NCORES)], axis=0)
    return out.astype(np.float32).reshape(B, NPAIR, D)


# revision 3
# speedup vs baseline: 1.6762x; 1.0693x over previous
"""Pairwise-interaction kernel for Trainium2 (raw Bass), 8-core SPMD.

Computes out[b, p, :] = x[b, i(p), :] * x[b, j(p), :] for all pairs
(i < j) of the F=26 feature rows, p ordered row-major (i outer, j inner).

Sharding: data-parallel over the batch dim (16384 -> 8 x 2048), no
cross-core communication.

v3 design notes:
  * All tensors bf16: DVE 2x packing mode doubles tensor_tensor
    throughput vs f32 AND halves HBM traffic. The added rounding error
    (two input roundings + one output rounding, ~1.2% worst case) is
    well inside the 2e-2 relative-error gate; f32<->bf16 conversion
    happens on the host.
  * G=4 samples share each partition row (sample = t*P*G + p*G + g), so
    every TT instruction covers G groups (amortizes the ~58-cycle DVE
    per-instruction bubble) and, because the G samples are CONSECUTIVE
    rows of x/y, each DMA descriptor row moves G*20.8KB of contiguous
    DRAM - the v2 trace showed ~5KB descriptor rows were
    packet-overhead-bound (338 GB/s effective).
  * One store DMA per ring per supertile (sync ring: groups 0..1,
    scalar ring: groups 2..3), overlapped with the next supertile's
    compute via YB=2 output buffers.

Raw-Bass sync scheme (every instruction carries at most ONE semaphore
wait; extra ordering uses standalone wait_ge ops on the engine queue):
  sem_ld   (+16 per load DMA, scalar ring)
  sem_st_s (+16 per store DMA, sync ring)
  sem_st_a (+16 per store DMA, scalar ring)
  sem_tt   (+1 by the last TT of each supertile, vector engine)
"""

import numpy as np
import ml_dtypes

import concourse.bass as bass
from concourse import mybir
from concourse.bass_utils import run_bass_kernel_spmd

B, F, D = 16384, 26, 32
NCORES = 8
BC = B // NCORES           # 2048 samples per core
P = 128                    # SBUF partitions
G = 4                      # sample groups per supertile (consecutive rows)
NTS = BC // (P * G)        # 4 supertiles per core
FD = F * D                 # 832
NPAIR = F * (F - 1) // 2   # 325
OD = NPAIR * D             # 10400

XB = 3                     # input supertile buffers
YB = 2                     # output supertile buffers

# ring split of each supertile's store: columns [0, GS*OD) -> sync ring,
# [GS*OD, G*OD) -> scalar ring
GS = G // 2

BF16 = mybir.dt.bfloat16
NP_BF16 = ml_dtypes.bfloat16


def _pair_off(i_lo):
    return sum(F - 1 - i for i in range(i_lo))


_nc_cache = None


def _build_nc():
    nc = bass.Bass()
    x = nc.declare_dram_parameter("x", [BC, FD], BF16, isOutput=False)
    y = nc.declare_dram_parameter("y", [BC, OD], BF16, isOutput=True)
    # sample s = t*P*G + p*G + g: partition p's G samples are consecutive
    # DRAM rows, so per-partition DMA runs are G*FD / G*OD contiguous.
    xv = x[:].rearrange("(t p g) m -> t p (g m)", p=P, g=G)
    yv = y[:].rearrange("(t p g) m -> t p (g m)", p=P, g=G)

    with (
        nc.sbuf_tensor([P, XB * G * FD], BF16) as xbuf,
        nc.sbuf_tensor([P, YB * G * OD], BF16) as ybuf,
        nc.semaphore("sem_ld") as sem_ld,
        nc.semaphore("sem_st_s") as sem_st_s,
        nc.semaphore("sem_st_a") as sem_st_a,
        nc.semaphore("sem_tt") as sem_tt,
        nc.Block() as blk,
    ):
        xts = [xbuf[:, b * G * FD : (b + 1) * G * FD] for b in range(XB)]
        yts = [ybuf[:, b * G * OD : (b + 1) * G * OD] for b in range(YB)]

        def store(eng, t, lo, hi):
            # store columns [lo*OD, hi*OD) (groups lo..hi) of supertile t
            st = eng.dma_start(
                yv[t][:, lo * OD : hi * OD],
                yts[t % YB][:, lo * OD : hi * OD],
            )
            st._wait_ge(sem_tt, t + 1)
            return st

        @blk.scalar
        def _(scalar):
            for t in range(NTS):
                if t >= XB:
                    # slot free once the previous tenant's TTs have read it
                    scalar.wait_ge(sem_tt, t - XB + 1)
                scalar.dma_start(xts[t % XB], xv[t]).then_inc(sem_ld, 16)
                if t > 0:
                    store(scalar, t - 1, GS, G).then_inc(sem_st_a, 16)
            store(scalar, NTS - 1, GS, G).then_inc(sem_st_a, 16)

        @blk.sync
        def _(sync):
            for t in range(NTS):
                store(sync, t, 0, GS).then_inc(sem_st_s, 16)

        @blk.vector
        def _(v):
            for t in range(NTS):
                xt = xts[t % XB].rearrange("p (g m) -> p g m", g=G)
                yt = yts[t % YB].rearrange("p (g m) -> p g m", g=G)
                v.wait_ge(sem_ld, 16 * (t + 1))
                if t >= YB:
                    # ybuf slot free once both rings stored supertile t-YB
                    v.wait_ge(sem_st_s, 16 * (t - YB + 1))
                    v.wait_ge(sem_st_a, 16 * (t - YB + 1))
                off = 0
                for i in range(F - 1):
                    nrep = F - 1 - i
                    in0 = (
                        xt[:, :, i * D : (i + 1) * D]
                        .unsqueeze(2)
                        .broadcast_to([P, G, nrep, D])
                    )
                    in1 = xt[:, :, (i + 1) * D : FD].rearrange(
                        "p g (r d) -> p g r d", d=D
                    )
                    outap = yt[:, :, off * D : (off + nrep) * D].rearrange(
                        "p g (r d) -> p g r d", d=D
                    )
                    tt = nc.vector.tensor_mul(outap, in0, in1)
                    off += nrep
                tt.then_inc(sem_tt, 1)

    return nc


def _make_in_maps(inputs: np.ndarray):
    x = np.asarray(inputs, dtype=np.float32).reshape(B, FD).astype(NP_BF16)
    shards = np.ascontiguousarray(x.reshape(NCORES, BC, FD))
    return [{"x": shards[c]} for c in range(NCORES)]


def kernel(inputs: np.ndarray) -> np.ndarray:
    global _nc_cache
    if _nc_cache is None:
        _nc_cache = _build_nc()
    nc = _nc_cache

    in_maps = _make_in_maps(inputs)
    res = run_bass_kernel_spmd(nc, in_maps, list(range(NCORES)))
    out = np.concatenate([res.results[c]["y"] for c in range(NCORES)], axis=0)
    return out.astype(np.float32).reshape(B, NPAIR, D)


# revision 4
# speedup vs baseline: 1.9134x; 1.1416x over previous
"""Pairwise-interaction kernel for Trainium2 (raw Bass), 8-core SPMD.

Computes out[b, p, :] = x[b, i(p), :] * x[b, j(p), :] for all pairs
(i < j) of the F=26 feature rows, p ordered row-major (i outer, j inner).

Sharding: data-parallel over the batch dim (16384 -> 8 x 2048), no
cross-core communication.

v4 design notes:
  * All tensors bf16: DVE 2x packing mode doubles tensor_tensor
    throughput vs f32 AND halves HBM traffic (core DMA sustains only
    ~440 GB/s combined; output is 42.6MB/core in bf16). The added
    rounding error (~1.2% worst case) is well inside the 2e-2 gate;
    f32<->bf16 conversion happens on the host.
  * Samples are interleaved G=4 per partition row (sample =
    t*P*G + p*G + g) so TT instructions cover multiple groups
    (amortizing the ~58-cycle DVE bubble) and every DMA descriptor row
    is 20.8-83.2KB of contiguous DRAM (small rows are packet-bound).
  * All NTS=4 input loads are issued up-front on the scalar ring
    (XB=NTS) so no load ever queues behind a multi-MB store (the v3
    trace showed a 23us DVE stall from exactly that).
  * Each supertile's 25-TT sweep runs twice over half the groups
    (g 0..1 then 2..3); each half is stored immediately, halves
    alternating between the sync and scalar HWDGE rings, so stores
    overlap compute and the drain tail is a half-supertile.

Raw-Bass sync scheme (one semaphore wait per instruction; extra
ordering uses standalone wait_ge ops on the engine queue):
  sem_ld   (+16 per load DMA, scalar ring)
  sem_st_s (+16 per store DMA, sync ring: first half of each supertile)
  sem_st_a (+16 per store DMA, scalar ring: second half)
  sem_tt   (+1 by the last TT of each half-sweep, vector engine)
"""

import numpy as np
import ml_dtypes

import concourse.bass as bass
from concourse import mybir
from concourse.bass_utils import run_bass_kernel_spmd

B, F, D = 16384, 26, 32
NCORES = 8
BC = B // NCORES           # 2048 samples per core
P = 128                    # SBUF partitions
G = 4                      # sample groups per supertile (consecutive rows)
GS = G // 2                # groups per half-sweep
NTS = BC // (P * G)        # 4 supertiles per core
FD = F * D                 # 832
NPAIR = F * (F - 1) // 2   # 325
OD = NPAIR * D             # 10400

XB = NTS                   # all input supertiles resident at once
YB = 2                     # output supertile buffers

BF16 = mybir.dt.bfloat16
NP_BF16 = ml_dtypes.bfloat16

_nc_cache = None


def _build_nc():
    nc = bass.Bass()
    x = nc.declare_dram_parameter("x", [BC, FD], BF16, isOutput=False)
    y = nc.declare_dram_parameter("y", [BC, OD], BF16, isOutput=True)
    # sample s = t*P*G + p*G + g: partition p's G samples are consecutive
    # DRAM rows, so per-partition DMA runs are long and contiguous.
    xv = x[:].rearrange("(t p g) m -> t p (g m)", p=P, g=G)
    yv = y[:].rearrange("(t p g) m -> t p (g m)", p=P, g=G)

    with (
        nc.sbuf_tensor([P, XB * G * FD], BF16) as xbuf,
        nc.sbuf_tensor([P, YB * G * OD], BF16) as ybuf,
        nc.semaphore("sem_ld") as sem_ld,
        nc.semaphore("sem_st_s") as sem_st_s,
        nc.semaphore("sem_st_a") as sem_st_a,
        nc.semaphore("sem_tt") as sem_tt,
        nc.Block() as blk,
    ):
        xts = [xbuf[:, b * G * FD : (b + 1) * G * FD] for b in range(XB)]
        yts = [ybuf[:, b * G * OD : (b + 1) * G * OD] for b in range(YB)]

        def store(eng, t, h):
            # store groups [h*GS, (h+1)*GS) of supertile t; ready when
            # half-sweep 2t+h+1 has retired
            st = eng.dma_start(
                yv[t][:, h * GS * OD : (h + 1) * GS * OD],
                yts[t % YB][:, h * GS * OD : (h + 1) * GS * OD],
            )
            st._wait_ge(sem_tt, 2 * t + h + 1)
            return st

        @blk.scalar
        def _(scalar):
            for t in range(NTS):
                scalar.dma_start(xts[t], xv[t]).then_inc(sem_ld, 16)
            for t in range(NTS):
                store(scalar, t, 1).then_inc(sem_st_a, 16)

        @blk.sync
        def _(sync):
            for t in range(NTS):
                store(sync, t, 0).then_inc(sem_st_s, 16)

        @blk.vector
        def _(v):
            for t in range(NTS):
                xt = xts[t].rearrange("p (g m) -> p g m", g=G)
                yt = yts[t % YB].rearrange("p (g m) -> p g m", g=G)
                v.wait_ge(sem_ld, 16 * (t + 1))
                for h in range(2):
                    if t >= YB:
                        # this half of ybuf slot t-YB has been stored
                        sem = sem_st_s if h == 0 else sem_st_a
                        v.wait_ge(sem, 16 * (t - YB + 1))
                    xh = xt[:, h * GS : (h + 1) * GS]
                    yh = yt[:, h * GS : (h + 1) * GS]
                    off = 0
                    for i in range(F - 1):
                        nrep = F - 1 - i
                        in0 = (
                            xh[:, :, i * D : (i + 1) * D]
                            .unsqueeze(2)
                            .broadcast_to([P, GS, nrep, D])
                        )
                        in1 = xh[:, :, (i + 1) * D : FD].rearrange(
                            "p g (r d) -> p g r d", d=D
                        )
                        outap = yh[
                            :, :, off * D : (off + nrep) * D
                        ].rearrange("p g (r d) -> p g r d", d=D)
                        tt = nc.vector.tensor_mul(outap, in0, in1)
                        off += nrep
                    tt.then_inc(sem_tt, 1)

    return nc


def _make_in_maps(inputs: np.ndarray):
    x = np.asarray(inputs, dtype=np.float32).reshape(B, FD).astype(NP_BF16)
    shards = np.ascontiguousarray(x.reshape(NCORES, BC, FD))
    return [{"x": shards[c]} for c in range(NCORES)]


def kernel(inputs: np.ndarray) -> np.ndarray:
    global _nc_cache
    if _nc_cache is None:
        _nc_cache = _build_nc()
    nc = _nc_cache

    in_maps = _make_in_maps(inputs)
    res = run_bass_kernel_spmd(nc, in_maps, list(range(NCORES)))
    out = np.concatenate([res.results[c]["y"] for c in range(NCORES)], axis=0)
    return out.astype(np.float32).reshape(B, NPAIR, D)


# revision 5
# speedup vs baseline: 2.0478x; 1.0702x over previous
"""Pairwise-interaction kernel for Trainium2 (raw Bass), 8-core SPMD.

Computes out[b, p, :] = x[b, i(p), :] * x[b, j(p), :] for all pairs
(i < j) of the F=26 feature rows, p ordered row-major (i outer, j inner).

Sharding: data-parallel over the batch dim (16384 -> 8 x 2048), no
cross-core communication.

v5 design notes:
  * All tensors bf16: DVE 2x packing mode doubles tensor_tensor
    throughput vs f32 AND halves HBM traffic. The added rounding error
    (~1.2% worst case) is well inside the 2e-2 gate; f32<->bf16
    conversion happens on the host.
  * Samples are interleaved G=4 per partition row (sample =
    t*P*G + p*G + g): every TT instruction covers all 4 groups
    (amortizes the ~58-cycle DVE bubble; DVE ~93.5us total) and DMA
    descriptor rows are multi-KB contiguous DRAM runs.
  * The exec floor is the store stream: first-chunk-ready +
    42.6MB / ~430GB/s (16 SDMA engines x ~27GB/s, shared by both HWDGE
    rings - so all stores ride ONE ring and rings only matter for FIFO
    isolation). Chunks are pair-ranges of each supertile's sweep,
    sized small-big-big-small so the stream starts ~3us into the first
    sweep and the post-compute tail is only ~2us.
  * All NTS=4 input loads are issued up-front on the scalar ring so no
    load ever queues behind a multi-MB store.

Raw-Bass sync scheme (one semaphore wait per instruction; extra
ordering uses standalone wait_ge ops on the engine queue):
  sem_ld (+16 per load DMA, scalar ring)
  sem_st (+16 per store DMA, sync ring; 4 chunk-stores per supertile)
  sem_tt (+1 by the last TT of each chunk, vector engine)
"""

import numpy as np
import ml_dtypes

import concourse.bass as bass
from concourse import mybir
from concourse.bass_utils import run_bass_kernel_spmd

B, F, D = 16384, 26, 32
NCORES = 8
BC = B // NCORES           # 2048 samples per core
P = 128                    # SBUF partitions
G = 4                      # sample groups per supertile (consecutive rows)
NTS = BC // (P * G)        # 4 supertiles per core
FD = F * D                 # 832
NPAIR = F * (F - 1) // 2   # 325
OD = NPAIR * D             # 10400

XB = NTS                   # all input supertiles resident at once
YB = 2                     # output supertile buffers

# i-block ranges per store chunk: pair counts (49, 171, 77, 28) -
# small first chunk so the store stream starts early, small last chunk
# so the post-compute drain is short.
CHUNKS = [(0, 2), (2, 10), (10, 18), (18, 25)]
NCH = len(CHUNKS)

BF16 = mybir.dt.bfloat16
NP_BF16 = ml_dtypes.bfloat16


def _pair_off(i_lo):
    return sum(F - 1 - i for i in range(i_lo))


_nc_cache = None


def _build_nc():
    nc = bass.Bass()
    x = nc.declare_dram_parameter("x", [BC, FD], BF16, isOutput=False)
    y = nc.declare_dram_parameter("y", [BC, OD], BF16, isOutput=True)
    # sample s = t*P*G + p*G + g: partition p's G samples are consecutive
    # DRAM rows, so per-partition DMA runs are long and contiguous.
    xv = x[:].rearrange("(t p g) m -> t p (g m)", p=P, g=G)
    yv = y[:].rearrange("(t p g) m -> t p g m", p=P, g=G)

    with (
        nc.sbuf_tensor([P, XB * G * FD], BF16) as xbuf,
        nc.sbuf_tensor([P, YB * G * OD], BF16) as ybuf,
        nc.semaphore("sem_ld") as sem_ld,
        nc.semaphore("sem_st") as sem_st,
        nc.semaphore("sem_tt") as sem_tt,
        nc.Block() as blk,
    ):
        xts = [xbuf[:, b * G * FD : (b + 1) * G * FD] for b in range(XB)]
        yts = [ybuf[:, b * G * OD : (b + 1) * G * OD] for b in range(YB)]

        @blk.scalar
        def _(scalar):
            for t in range(NTS):
                scalar.dma_start(xts[t], xv[t]).then_inc(sem_ld, 16)

        @blk.sync
        def _(sync):
            for t in range(NTS):
                yt = yts[t % YB].rearrange("p (g m) -> p g m", g=G)
                for c, (i_lo, i_hi) in enumerate(CHUNKS):
                    p_lo, p_hi = _pair_off(i_lo), _pair_off(i_hi)
                    st = sync.dma_start(
                        yv[t][:, :, p_lo * D : p_hi * D],
                        yt[:, :, p_lo * D : p_hi * D],
                    )
                    st._wait_ge(sem_tt, NCH * t + c + 1)
                    st.then_inc(sem_st, 16)

        @blk.vector
        def _(v):
            for t in range(NTS):
                xt = xts[t].rearrange("p (g m) -> p g m", g=G)
                yt = yts[t % YB].rearrange("p (g m) -> p g m", g=G)
                v.wait_ge(sem_ld, 16 * (t + 1))
                for c, (i_lo, i_hi) in enumerate(CHUNKS):
                    if t >= YB:
                        # chunk c of ybuf slot t-YB has been stored
                        v.wait_ge(sem_st, 16 * (NCH * (t - YB) + c + 1))
                    off = _pair_off(i_lo)
                    for i in range(i_lo, i_hi):
                        nrep = F - 1 - i
                        in0 = (
                            xt[:, :, i * D : (i + 1) * D]
                            .unsqueeze(2)
                            .broadcast_to([P, G, nrep, D])
                        )
                        in1 = xt[:, :, (i + 1) * D : FD].rearrange(
                            "p g (r d) -> p g r d", d=D
                        )
                        outap = yt[
                            :, :, off * D : (off + nrep) * D
                        ].rearrange("p g (r d) -> p g r d", d=D)
                        tt = nc.vector.tensor_mul(outap, in0, in1)
                        off += nrep
                    tt.then_inc(sem_tt, 1)

    return nc


def _make_in_maps(inputs: np.ndarray):
    x = np.asarray(inputs, dtype=np.float32).reshape(B, FD).astype(NP_BF16)
    shards = np.ascontiguousarray(x.reshape(NCORES, BC, FD))
    return [{"x": shards[c]} for c in range(NCORES)]


def kernel(inputs: np.ndarray) -> np.ndarray:
    global _nc_cache
    if _nc_cache is None:
        _nc_cache = _build_nc()
    nc = _nc_cache

    in_maps = _make_in_maps(inputs)
    res = run_bass_kernel_spmd(nc, in_maps, list(range(NCORES)))
    out = np.concatenate([res.results[c]["y"] for c in range(NCORES)], axis=0)
    return out.astype(np.float32).reshape(B, NPAIR, D)
